# revision 25
# baseline (speedup 1.0000x reference)
"""Trainium2 Bass kernel for GQA attention with QK-RMSNorm, RoPE and a
bidirectional-prefix + causal mask (sparse_attention problem).

Reference computation (fp32):
  xq = x @ wq.T; xk = x @ wk.T; xv = x @ wv.T   (per-head RMSNorm on q,k)
  rope(q), rope(k); repeat kv heads 8x
  scores = q k^T / sqrt(128); mask = causal OR (i<p & j<p)
  out = softmax(scores) @ v;  y = out @ wo.T

Sharding: 8 cores = 2 batches x 4 head-groups (4 query heads each, sharing
one KV head).  Each core computes a partial y^T (its 4 heads' contribution);
the host sums the 4 partials per batch and transposes back.

v2 design (vs the 547us baseline):
  * All projections computed TRANSPOSED (feature-on-partition) directly:
    qT[hd, tok] = wqT^T @ xT per 128-col head slice -- no PE transposes of
    q/k, no PSUM->SBUF roundtrip of token-major q.
  * bf16 everywhere on SBUF (halves DMA, DVE 2x, cheap LDWEIGHTS); PSUM
    accumulation stays fp32.
  * RMSNorm in transposed layout: Sum(q^2) over the head dim (=partitions)
    via an all-ones [128,128] matmul -> the result is broadcast across
    partitions for free; rsqrt = Act Sqrt + DVE fast reciprocal.
    softmax 1/sqrt(HD) folded into the q norm factor.
  * Sparse masking by SUFFIX-RANGED matmuls: per 128-k-block only the
    query columns whose extent covers the block are computed (exact 137
    of 256 blocks); the causal diagonal block mask is ADDED BY A SECOND
    MATMUL (lhsT=dmask^T, rhs=identity) accumulating into the same PSUM.
  * Softmax denominators via all-ones matmul accumulated per k-block
    (fp32, broadcast across partitions) -- no [1,512] slow ops, no DRAM
    broadcast roundtrip.
  * Static emission interleave keeps the PE queue dense: scores of unit
    (g,h) interleave with Z/AV chains of (g,h-1) and the WO of group g-1.

TRN2 ISA allows ONE sync-wait per instruction and walrus does not split
multi-wait instructions, so `_legalize_waits` rewrites the emitted BIR,
moving excess waits onto preceding same-engine NoOps.
"""
import math
import numpy as np
from contextlib import ExitStack

import ml_dtypes
import bass_rust
import concourse.bass as bass
import concourse.mybir as mybir
import concourse.tile as tile
from concourse.bass_utils import run_bass_kernel_spmd
from concourse.masks import make_identity

F32 = mybir.dt.float32
BF16 = mybir.dt.bfloat16
AF = mybir.ActivationFunctionType

B, S, D = 2, 2048, 2048
NH, KVH, HD = 16, 2, 128
HPC = 4                      # query heads per core
N_CORES = 8
EPS = 1e-6
NEG = -1.0e30

SB = S // 128                # 16 token blocks
DB = D // 128                # 16 contraction blocks
GS = 512                     # tokens per group
NG = S // GS                 # 4 groups

_lgw_counter = [0]


def _legalize_waits(nc, cap=1):
    """Move all-but-`cap` sync waits of every instruction onto preceding
    same-engine NoOps (TRN2 EVENTS block has a single wait slot)."""
    for fn in nc.m.functions:
        for blk in fn.blocks:
            out = []
            changed = False
            for inst in blk.instructions:
                si = inst.sync_info
                waits = list(si.on_wait) if si is not None and si.on_wait else []
                if len(waits) > cap:
                    changed = True
                    move, keep = waits[:-cap], waits[-cap:]
                    for w in move:
                        n = bass_rust.InstNoOp(name=f"LGW-{_lgw_counter[0]}")
                        _lgw_counter[0] += 1
                        n.engine = inst.engine
                        n.sync_info = mybir.SyncInfo(on_wait=[w], on_update=[])
                        out.append(n)
                    inst.sync_info = mybir.SyncInfo(
                        on_wait=keep, on_update=list(si.on_update or []))
                out.append(inst)
            if changed:
                blk.instructions = out
    return nc


def _eblks(p):
    """Key extent (in 128-blocks) attended by each query row-block."""
    out = []
    for rb in range(SB):
        hi = (rb + 1) * 128
        out.append((p if hi <= p else hi) // 128)
    return out


def _group_info(p):
    """Per group: (gmax, sfx[kb], diag[kb]).  sfx = start column (within the
    512-token group) of the query suffix that attends k-block kb; diag =
    whether kb is the causal diagonal of some row-block (always at suffix
    position 0)."""
    ebl = _eblks(p)
    infos = []
    for g in range(NG):
        eb = [ebl[rb] for rb in range(g * 4, g * 4 + 4)]
        gmax = max(eb)
        sfx, diag = [], []
        for kb in range(gmax):
            jm = sum(1 for e in eb if e <= kb)
            sfx.append(jm * 128)
            dg = False
            for i, rb in enumerate(range(g * 4, g * 4 + 4)):
                if eb[i] == kb + 1 and rb * 128 >= p:
                    assert i == jm, "diagonal must sit at suffix position 0"
                    dg = True
            diag.append(dg)
        infos.append((gmax, sfx, diag))
    return infos


def build_core_kernel(p, legalize=True):
    """One SPMD program; per-core behavior differs only via input data."""
    nc = bass.Bass()

    xT = nc.dram_tensor("xT", [D, S], BF16, kind="ExternalInput")
    wqT = nc.dram_tensor("wqT", [D, HPC * HD], BF16, kind="ExternalInput")
    wkT = nc.dram_tensor("wkT", [D, HD], BF16, kind="ExternalInput")
    wvT = nc.dram_tensor("wvT", [D, HD], BF16, kind="ExternalInput")
    woT = nc.dram_tensor("woT", [HPC * HD, D], BF16, kind="ExternalInput")
    cos_q = nc.dram_tensor("cos_q", [HD, S], BF16, kind="ExternalInput")
    sin_q = nc.dram_tensor("sin_q", [HD, S], BF16, kind="ExternalInput")
    cos_k = nc.dram_tensor("cos_k", [HD, S], BF16, kind="ExternalInput")
    sin_k = nc.dram_tensor("sin_k", [HD, S], BF16, kind="ExternalInput")
    dmaskT = nc.dram_tensor("dmaskT", [128, 128], BF16, kind="ExternalInput")
    rotT = nc.dram_tensor("rotT", [128, 128], BF16, kind="ExternalInput")
    yT = nc.dram_tensor("yT", [D, S], F32, kind="ExternalOutput")

    infos = _group_info(p)
    h2 = HD // 2

    with tile.TileContext(nc) as tc, ExitStack() as octx:
        const = octx.enter_context(tc.tile_pool(name="const", bufs=1))
        ident = const.tile([128, 128], BF16)
        make_identity(nc, ident)
        ones_b = const.tile([128, 128], BF16)
        nc.vector.memset(ones_b, 1.0)
        ones_r = const.tile([128, 128], mybir.dt.float32r)
        nc.vector.memset(ones_r, 1.0)
        eps_q_t = const.tile([128, 1], F32)
        nc.vector.memset(eps_q_t, HD * EPS)
        eps_k_t = const.tile([128, 1], F32)
        nc.vector.memset(eps_k_t, EPS)
        zero_t = const.tile([128, 1], F32)
        nc.vector.memset(zero_t, 0.0)
        dmask_sb = const.tile([128, 128], BF16)
        rot_sb = const.tile([128, 128], BF16)
        nc.scalar.dma_start(out=rot_sb, in_=rotT[:, :])

        # weights + trig on the Activation hwdge queue (x / y use SP's)
        wpool = octx.enter_context(tc.tile_pool(name="w", bufs=1))
        wq_sb = wpool.tile([128, DB, HPC * HD], BF16)
        wk_sb = wpool.tile([128, DB, HD], BF16)
        wv_sb = wpool.tile([128, DB, HD], BF16)
        wo_sb = wpool.tile([128, HPC, D], BF16)
        cq_sb = wpool.tile([128, S], BF16)
        sq_sb = wpool.tile([128, S], BF16)
        ck_sb = wpool.tile([128, S], BF16)
        sk_sb = wpool.tile([128, S], BF16)
        nc.scalar.dma_start(
            out=wq_sb, in_=wqT.rearrange("(kb pp) m -> pp kb m", pp=128))
        nc.scalar.dma_start(
            out=wk_sb, in_=wkT.rearrange("(kb pp) m -> pp kb m", pp=128))
        nc.scalar.dma_start(
            out=wv_sb, in_=wvT.rearrange("(kb pp) m -> pp kb m", pp=128))
        nc.scalar.dma_start(out=cq_sb, in_=cos_q[:, :])
        nc.scalar.dma_start(out=sq_sb, in_=sin_q[:, :])
        nc.scalar.dma_start(out=ck_sb, in_=cos_k[:, :])
        nc.scalar.dma_start(out=sk_sb, in_=sin_k[:, :])
        nc.scalar.dma_start(out=dmask_sb, in_=dmaskT[:, :])
        nc.scalar.dma_start(
            out=wo_sb, in_=woT.rearrange("(hb pp) d -> pp hb d", pp=128))

        persist = octx.enter_context(tc.tile_pool(name="persist", bufs=1))
        qTn = persist.tile([128, HPC, S], BF16)   # [hd, h, tok]
        kTn = persist.tile([128, S], BF16)        # [hd, tok]
        v_all = persist.tile([128, SB, HD], BF16)  # [tok(P), kb, hd]

        # ---------------- Phase 1: transposed projections ------------------
        with tc.tile_pool(name="p1x", bufs=2) as p1x, \
             tc.tile_pool(name="p1", bufs=3) as p1, \
             tc.tile_pool(name="p1v", bufs=2) as p1v, \
             tc.tile_pool(name="proj_ps", bufs=3, space="PSUM") as proj_ps, \
             tc.tile_pool(name="zn_ps", bufs=2, space="PSUM") as zn_ps, \
             tc.tile_pool(name="vt_ps", bufs=1, space="PSUM") as vt_ps:

            def proj_chain(w_sb, h, xg):
                ps = proj_ps.tile([128, GS], F32, tag="proj")
                for kb in range(DB):
                    nc.tensor.matmul(
                        ps, lhsT=w_sb[:, kb, h * HD:(h + 1) * HD],
                        rhs=xg[:, kb, :], start=(kb == 0), stop=(kb == DB - 1))
                return ps

            def norm_rope(src_ps, cosv, sinv, dst, sc, bi):
                """RMSNorm + RoPE in [hd, tok] layout; dst bf16 [128, GS].
                Norm factor r = 1/sqrt(sc*Z + bi) broadcast over partitions
                via all-ones matmul (Z = sum_hd q^2).  rotate_half is a PE
                matmul with a signed permutation matrix (DVE cannot read
                partition-shifted operands)."""
                sq = p1.tile([128, GS], BF16, tag="sq")
                nc.scalar.activation(out=sq, in_=src_ps, func=AF.Square,
                                     bias=zero_t)
                zz = zn_ps.tile([128, GS], F32, tag="zz")
                nc.tensor.matmul(zz, lhsT=ones_b, rhs=sq, start=True, stop=True)
                # r = (sc*Z + bi)^(-1/2) = exp(-0.5*ln(sc*Z + bi)) on Act --
                # ln/exp share one table set; DVE reciprocal is ~3.4us/tile.
                lg = p1.tile([128, GS], F32, tag="lg")
                nc.scalar.activation(out=lg, in_=zz, func=AF.Ln,
                                     bias=bi, scale=sc)
                r = p1.tile([128, GS], F32, tag="r")
                nc.scalar.activation(out=r, in_=lg, func=AF.Exp,
                                     bias=zero_t, scale=-0.5)
                qs = p1.tile([128, GS], BF16, tag="qs")
                nc.scalar.copy(out=qs, in_=src_ps)
                rot = zn_ps.tile([128, GS], F32, tag="rot")
                nc.tensor.matmul(rot, lhsT=rot_sb, rhs=qs,
                                 start=True, stop=True)
                t1 = p1.tile([128, GS], BF16, tag="t1")
                nc.vector.tensor_mul(t1, qs, cosv)
                t2 = p1.tile([128, GS], BF16, tag="t2")
                nc.vector.tensor_mul(t2, rot, sinv)
                cb = p1.tile([128, GS], BF16, tag="cb")
                nc.vector.tensor_add(cb, t1, t2)
                nc.vector.tensor_mul(dst, cb, r)

            for g in range(NG):
                ts = slice(g * GS, (g + 1) * GS)
                xg = p1x.tile([128, DB, GS], BF16, tag="xg")
                if g == 0:
                    # split the first load so the first proj chain starts
                    # as soon as the leading half lands
                    hD = D // 2
                    nc.sync.dma_start(
                        out=xg[:, 0:DB // 2, :],
                        in_=xT[0:hD, ts].rearrange(
                            "(kb pp) t -> pp kb t", pp=128))
                    nc.sync.dma_start(
                        out=xg[:, DB // 2:DB, :],
                        in_=xT[hD:D, ts].rearrange(
                            "(kb pp) t -> pp kb t", pp=128))
                else:
                    nc.sync.dma_start(
                        out=xg,
                        in_=xT[:, ts].rearrange("(kb pp) t -> pp kb t", pp=128))

                # pipelined emission: chain h+1 before norm/rope of chain h
                ps_q = [None] * HPC
                ps_q[0] = proj_chain(wq_sb, 0, xg)
                ps_q[1] = proj_chain(wq_sb, 1, xg)
                norm_rope(ps_q[0], cq_sb[:, ts], sq_sb[:, ts],
                          qTn[:, 0, ts], 1.0, eps_q_t)
                ps_q[2] = proj_chain(wq_sb, 2, xg)
                norm_rope(ps_q[1], cq_sb[:, ts], sq_sb[:, ts],
                          qTn[:, 1, ts], 1.0, eps_q_t)
                ps_q[3] = proj_chain(wq_sb, 3, xg)
                norm_rope(ps_q[2], cq_sb[:, ts], sq_sb[:, ts],
                          qTn[:, 2, ts], 1.0, eps_q_t)
                ps_k = proj_chain(wk_sb, 0, xg)
                norm_rope(ps_q[3], cq_sb[:, ts], sq_sb[:, ts],
                          qTn[:, 3, ts], 1.0, eps_q_t)
                ps_v = proj_chain(wv_sb, 0, xg)
                norm_rope(ps_k, ck_sb[:, ts], sk_sb[:, ts],
                          kTn[:, ts], 1.0 / HD, eps_k_t)

                # v: cast to bf16 + PE transpose into [tok, hd]
                vs = p1v.tile([128, GS], BF16, tag="vs")
                nc.scalar.copy(out=vs, in_=ps_v)
                for j in range(4):
                    vt = vt_ps.tile([128, 64], F32, tag="vt")
                    vt_b = vt.bitcast(BF16)
                    nc.tensor.transpose(
                        vt_b, vs[:, j * 128:(j + 1) * 128], ident)
                    nc.vector.tensor_copy(
                        out=v_all[:, g * 4 + j, :], in_=vt_b)

        # ------------- Phase 2: attention + WO, interleaved ----------------
        with tc.tile_pool(name="p2m", bufs=3) as p2m, \
             tc.tile_pool(name="p2a", bufs=2) as p2a, \
             tc.tile_pool(name="p2e", bufs=2) as p2e, \
             tc.tile_pool(name="p2y", bufs=3) as p2y, \
             tc.tile_pool(name="s_ps", bufs=3, space="PSUM") as s_psp, \
             tc.tile_pool(name="z_ps", bufs=1, space="PSUM") as z_psp, \
             tc.tile_pool(name="av_ps", bufs=2, space="PSUM") as av_psp, \
             tc.tile_pool(name="y_ps", bufs=2, space="PSUM") as y_psp:

            def sc_gen(g, h, expT):
                gmax, sfx, diag = infos[g]
                for kb in range(gmax):
                    a = sfx[kb]
                    sp = s_psp.tile([128, GS], F32, tag="s")
                    nc.tensor.matmul(
                        sp[:, 0:GS - a],
                        lhsT=kTn[:, kb * 128:(kb + 1) * 128],
                        rhs=qTn[:, h, g * GS + a:(g + 1) * GS],
                        start=True, stop=not diag[kb])
                    if diag[kb]:
                        nc.tensor.matmul(sp[:, 0:128], lhsT=dmask_sb,
                                         rhs=ident, start=False, stop=True)
                    nc.scalar.activation(out=expT[:, kb, a:GS],
                                         in_=sp[:, 0:GS - a], func=AF.Exp,
                                         bias=zero_t)
                    yield

            def zav_gen(g, h, expT, attnT):
                gmax, sfx, _ = infos[g]
                # sum exp over k-blocks on DVE (fp32, exact); one all-ones
                # matmul then reduces over partitions -> Z broadcast.
                es = p2m.tile([128, GS], F32, tag="es")
                if gmax > 1 and sfx[1] == 0:
                    nc.vector.tensor_add(es, expT[:, 0, :], expT[:, 1, :])
                    k0 = 2
                else:
                    nc.vector.tensor_copy(out=es, in_=expT[:, 0, :])
                    k0 = 1
                for kb in range(k0, gmax):
                    a = sfx[kb]
                    nc.vector.tensor_add(es[:, a:GS], es[:, a:GS],
                                         expT[:, kb, a:GS])
                    if kb % 2 == 0:
                        yield
                zp = z_psp.tile([128, GS], F32, tag="z")
                nc.tensor.matmul(zp, lhsT=ones_r,
                                 rhs=es.bitcast(mybir.dt.float32r),
                                 start=True, stop=True)
                yield
                ap = av_psp.tile([128, GS], F32, tag="av")
                for kb in range(gmax):
                    a = sfx[kb]
                    nc.tensor.matmul(ap[:, a:GS], lhsT=v_all[:, kb, :],
                                     rhs=expT[:, kb, a:GS],
                                     start=(kb == 0), stop=(kb == gmax - 1))
                    yield
                lz = p2m.tile([128, GS], F32, tag="lz")
                nc.scalar.activation(out=lz, in_=zp, func=AF.Ln,
                                     bias=zero_t)
                rz = p2m.tile([128, GS], F32, tag="rz")
                nc.scalar.activation(out=rz, in_=lz, func=AF.Exp,
                                     bias=zero_t, scale=-1.0)
                nc.vector.tensor_mul(attnT[:, h, :], ap, rz)
                yield

            def wo_gen(g, attnT):
                for db in range(DB):
                    yp = y_psp.tile([128, GS], F32, tag="y")
                    for hb in range(HPC):
                        nc.tensor.matmul(
                            yp, lhsT=wo_sb[:, hb, db * 128:(db + 1) * 128],
                            rhs=attnT[:, hb, :],
                            start=(hb == 0), stop=(hb == HPC - 1))
                    ys = p2y.tile([128, GS], F32, tag="ys")
                    if db % 2 == 0:
                        nc.vector.tensor_copy(out=ys, in_=yp)
                    else:
                        nc.scalar.copy(out=ys, in_=yp)
                    nc.sync.dma_start(
                        out=yT[db * 128:(db + 1) * 128, g * GS:(g + 1) * GS],
                        in_=ys)
                    yield

            def pull(gen, n):
                if gen is None:
                    return None
                for _ in range(n):
                    if next(gen, "END") == "END":
                        return None
                return gen

            def drain(gen):
                if gen is not None:
                    for _ in gen:
                        pass

            wo_bg = None
            for g in range(NG):
                attnT = p2a.tile([128, HPC, GS], BF16, tag="attnT")
                zav_bg = None
                for h in range(HPC):
                    expT = p2e.tile([128, SB, GS], BF16, tag="expT")
                    bg = wo_bg if h == 0 else zav_bg
                    for _ in sc_gen(g, h, expT):
                        bg = pull(bg, 1 if h == 0 else 3)
                    if h == 0:
                        wo_bg = bg      # keep leftover WO for the group tail
                    else:
                        drain(bg)
                    zav_bg = zav_gen(g, h, expT, attnT)
                # group tail: drain zav(h3) interleaved with leftover WO
                while zav_bg is not None or wo_bg is not None:
                    zav_bg = pull(zav_bg, 2)
                    wo_bg = pull(wo_bg, 1)
                wo_bg = wo_gen(g, attnT)
            drain(wo_bg)

    if legalize:
        _legalize_waits(nc)
    return nc


def _prep_inputs(x, cos, sin, wq, wk, wv, wo, q_gamma, k_gamma, p):
    """Build the 8 per-core input maps (all host-side prep is free)."""
    bf = ml_dtypes.bfloat16
    cos2 = np.asarray(cos, np.float32).reshape(S, HD)
    sin2 = np.asarray(sin, np.float32).reshape(S, HD)
    qg = np.asarray(q_gamma, np.float32)
    kg = np.asarray(k_gamma, np.float32)
    hh = HD // 2
    qg_rot = np.concatenate([qg[hh:], qg[:hh]])
    kg_rot = np.concatenate([kg[hh:], kg[:hh]])
    cosqT = np.ascontiguousarray((cos2 * qg).T.astype(bf))
    sinqT = np.ascontiguousarray((sin2 * qg_rot).T.astype(bf))
    coskT = np.ascontiguousarray((cos2 * kg).T.astype(bf))
    sinkT = np.ascontiguousarray((sin2 * kg_rot).T.astype(bf))

    ii = np.arange(128)
    dmask = np.where(ii[:, None] <= ii[None, :], 0.0, NEG).astype(np.float32)
    dmaskT = np.ascontiguousarray(dmask.T.astype(bf))

    # rotate_half as a signed permutation: rot = R @ q with
    # R[d, d+64] = -1 (d < 64), R[d, d-64] = +1 (d >= 64); lhsT = R^T.
    h2 = HD // 2
    R = np.zeros((HD, HD), np.float32)
    for dd in range(h2):
        R[dd, dd + h2] = -1.0
        R[dd + h2, dd] = 1.0
    rotT = np.ascontiguousarray(R.T.astype(bf))

    x = np.asarray(x, np.float32)
    wq = np.asarray(wq, np.float32)
    wk = np.asarray(wk, np.float32)
    wv = np.asarray(wv, np.float32)
    wo = np.asarray(wo, np.float32)

    xTb = [np.ascontiguousarray(x[b].T.astype(bf)) for b in range(B)]
    in_maps = []
    for c in range(N_CORES):
        b, gq = divmod(c, N_CORES // B)
        h0 = gq * HPC
        kv = h0 // (NH // KVH)
        in_maps.append({
            "xT": xTb[b],
            "wqT": np.ascontiguousarray(
                wq[h0 * HD:(h0 + HPC) * HD, :].T.astype(bf)),
            "wkT": np.ascontiguousarray(
                wk[kv * HD:(kv + 1) * HD, :].T.astype(bf)),
            "wvT": np.ascontiguousarray(
                wv[kv * HD:(kv + 1) * HD, :].T.astype(bf)),
            "woT": np.ascontiguousarray(
                wo[:, h0 * HD:(h0 + HPC) * HD].T.astype(bf)),
            "cos_q": cosqT, "sin_q": sinqT,
            "cos_k": coskT, "sin_k": sinkT,
            "dmaskT": dmaskT, "rotT": rotT,
        })
    return in_maps


def _gather(results):
    y = np.zeros((B, S, D), dtype=np.float32)
    for c in range(N_CORES):
        b = c // (N_CORES // B)
        y[b] += results[c]["yT"].T
    return y


def kernel(x, cos, sin, wq, wk, wv, wo, q_gamma, k_gamma, signal_token_num):
    p = int(signal_token_num)
    assert p % 128 == 0 and 0 <= p <= S, f"unsupported signal_token_num {p}"

    nc = build_core_kernel(p)
    in_maps = _prep_inputs(x, cos, sin, wq, wk, wv, wo, q_gamma, k_gamma, p)
    res = run_bass_kernel_spmd(nc, in_maps, list(range(N_CORES)))
    return _gather(res.results)


def _install_ntff_hook():
    """The container's antenv lacks axon_hooks; replicate the boot-time NTFF
    profile hook (ctypes into libaxon_pjrt.so) and register the module."""
    import sys
    import types
    import ctypes
    import contextlib

    if "antenv.axon_hooks" in sys.modules:
        return
    so_path = "/opt/axon/libaxon_pjrt.so"
    lib = ctypes.CDLL(so_path)
    if not hasattr(lib, "axon_start_nrt_profile"):
        return
    lib.axon_start_nrt_profile.argtypes = [
        ctypes.POINTER(ctypes.c_int64), ctypes.c_size_t]
    lib.axon_start_nrt_profile.restype = ctypes.c_int64
    lib.axon_stop_nrt_profile.argtypes = [ctypes.c_char_p]
    lib.axon_stop_nrt_profile.restype = ctypes.c_int64

    @contextlib.contextmanager
    def _hook(output_dir, device_ids):
        import jax
        jax.devices()
        if device_ids:
            ids = (ctypes.c_int64 * len(device_ids))(*device_ids)
            rc = lib.axon_start_nrt_profile(ids, len(device_ids))
        else:
            rc = lib.axon_start_nrt_profile(None, 0)
        if rc != 0:
            raise RuntimeError(f"axon_start_nrt_profile rc={rc}")
        try:
            yield
        finally:
            n = lib.axon_stop_nrt_profile(str(output_dir).encode())
            print(f"profile: {n} file(s) written to {output_dir}")

    import antenv
    mod = types.ModuleType("antenv.axon_hooks")
    mod.get_axon_ntff_profile_hook = lambda: _hook
    mod.set_axon_ntff_profile_hook = lambda h: None
    sys.modules["antenv.axon_hooks"] = mod
    antenv.axon_hooks = mod


def profile_once(inputs):
    """Run once with NTFF tracing; return max per-core exec time in ns."""
    import concourse.bass_utils as bu
    bu.upload_artifacts = lambda tmpdir: ""   # no bucket access here
    _install_ntff_hook()
    p = int(inputs["signal_token_num"])
    nc = build_core_kernel(p)
    in_maps = _prep_inputs(
        inputs["x"], inputs["cos"], inputs["sin"], inputs["wq"], inputs["wk"],
        inputs["wv"], inputs["wo"], inputs["q_gamma"], inputs["k_gamma"], p)
    try:
        res = bu.run_bass_kernel_spmd(nc, in_maps, list(range(N_CORES)),
                                      trace=True,
                                      trace_cores=list(range(N_CORES)))
        return res.exec_time_ns
    except Exception as e:
        print(f"profile failed: {type(e).__name__}: {e}")
        return None


# revision 27
# speedup vs baseline: 1.0189x; 1.0189x over previous
"""Trainium2 Bass kernel for GQA attention with QK-RMSNorm, RoPE and a
bidirectional-prefix + causal mask (sparse_attention problem).

Reference computation (fp32):
  xq = x @ wq.T; xk = x @ wk.T; xv = x @ wv.T   (per-head RMSNorm on q,k)
  rope(q), rope(k); repeat kv heads 8x
  scores = q k^T / sqrt(128); mask = causal OR (i<p & j<p)
  out = softmax(scores) @ v;  y = out @ wo.T

Sharding: 8 cores = 2 batches x 4 head-groups (4 query heads each, sharing
one KV head).  Each core computes a partial y^T (its 4 heads' contribution);
the host sums the 4 partials per batch and transposes back.

v2 design (vs the 547us baseline):
  * All projections computed TRANSPOSED (feature-on-partition) directly:
    qT[hd, tok] = wqT^T @ xT per 128-col head slice -- no PE transposes of
    q/k, no PSUM->SBUF roundtrip of token-major q.
  * bf16 everywhere on SBUF (halves DMA, DVE 2x, cheap LDWEIGHTS); PSUM
    accumulation stays fp32.
  * RMSNorm in transposed layout: Sum(q^2) over the head dim (=partitions)
    via an all-ones [128,128] matmul -> the result is broadcast across
    partitions for free; rsqrt = Act Sqrt + DVE fast reciprocal.
    softmax 1/sqrt(HD) folded into the q norm factor.
  * Sparse masking by SUFFIX-RANGED matmuls: per 128-k-block only the
    query columns whose extent covers the block are computed (exact 137
    of 256 blocks); the causal diagonal block mask is ADDED BY A SECOND
    MATMUL (lhsT=dmask^T, rhs=identity) accumulating into the same PSUM.
  * Softmax denominators via all-ones matmul accumulated per k-block
    (fp32, broadcast across partitions) -- no [1,512] slow ops, no DRAM
    broadcast roundtrip.
  * Static emission interleave keeps the PE queue dense: scores of unit
    (g,h) interleave with Z/AV chains of (g,h-1) and the WO of group g-1.

TRN2 ISA allows ONE sync-wait per instruction and walrus does not split
multi-wait instructions, so `_legalize_waits` rewrites the emitted BIR,
moving excess waits onto preceding same-engine NoOps.
"""
import math
import numpy as np
from contextlib import ExitStack

import ml_dtypes
import bass_rust
import concourse.bass as bass
import concourse.mybir as mybir
import concourse.tile as tile
from concourse.bass_utils import run_bass_kernel_spmd
from concourse.masks import make_identity

F32 = mybir.dt.float32
BF16 = mybir.dt.bfloat16
AF = mybir.ActivationFunctionType

B, S, D = 2, 2048, 2048
NH, KVH, HD = 16, 2, 128
HPC = 4                      # query heads per core
N_CORES = 8
EPS = 1e-6
NEG = -1.0e30

SB = S // 128                # 16 token blocks
DB = D // 128                # 16 contraction blocks
GS = 512                     # tokens per group
NG = S // GS                 # 4 groups

_lgw_counter = [0]


def _legalize_waits(nc, cap=1):
    """Move all-but-`cap` sync waits of every instruction onto preceding
    same-engine NoOps (TRN2 EVENTS block has a single wait slot)."""
    for fn in nc.m.functions:
        for blk in fn.blocks:
            out = []
            changed = False
            for inst in blk.instructions:
                si = inst.sync_info
                waits = list(si.on_wait) if si is not None and si.on_wait else []
                if len(waits) > cap:
                    changed = True
                    move, keep = waits[:-cap], waits[-cap:]
                    for w in move:
                        n = bass_rust.InstNoOp(name=f"LGW-{_lgw_counter[0]}")
                        _lgw_counter[0] += 1
                        n.engine = inst.engine
                        n.sync_info = mybir.SyncInfo(on_wait=[w], on_update=[])
                        out.append(n)
                    inst.sync_info = mybir.SyncInfo(
                        on_wait=keep, on_update=list(si.on_update or []))
                out.append(inst)
            if changed:
                blk.instructions = out
    return nc


def _eblks(p):
    """Key extent (in 128-blocks) attended by each query row-block."""
    out = []
    for rb in range(SB):
        hi = (rb + 1) * 128
        out.append((p if hi <= p else hi) // 128)
    return out


def _group_info(p):
    """Per group: (gmax, sfx[kb], diag[kb]).  sfx = start column (within the
    512-token group) of the query suffix that attends k-block kb; diag =
    whether kb is the causal diagonal of some row-block (always at suffix
    position 0)."""
    ebl = _eblks(p)
    infos = []
    for g in range(NG):
        eb = [ebl[rb] for rb in range(g * 4, g * 4 + 4)]
        gmax = max(eb)
        sfx, diag = [], []
        for kb in range(gmax):
            jm = sum(1 for e in eb if e <= kb)
            sfx.append(jm * 128)
            dg = False
            for i, rb in enumerate(range(g * 4, g * 4 + 4)):
                if eb[i] == kb + 1 and rb * 128 >= p:
                    assert i == jm, "diagonal must sit at suffix position 0"
                    dg = True
            diag.append(dg)
        infos.append((gmax, sfx, diag))
    return infos


def build_core_kernel(p, legalize=True):
    """One SPMD program; per-core behavior differs only via input data."""
    nc = bass.Bass()

    xT = nc.dram_tensor("xT", [D, S], BF16, kind="ExternalInput")
    wqT = nc.dram_tensor("wqT", [D, HPC * HD], BF16, kind="ExternalInput")
    wkT = nc.dram_tensor("wkT", [D, HD], BF16, kind="ExternalInput")
    wvT = nc.dram_tensor("wvT", [D, HD], BF16, kind="ExternalInput")
    woT = nc.dram_tensor("woT", [HPC * HD, D], BF16, kind="ExternalInput")
    cos_q = nc.dram_tensor("cos_q", [HD, S], BF16, kind="ExternalInput")
    sin_q = nc.dram_tensor("sin_q", [HD, S], BF16, kind="ExternalInput")
    cos_k = nc.dram_tensor("cos_k", [HD, S], BF16, kind="ExternalInput")
    sin_k = nc.dram_tensor("sin_k", [HD, S], BF16, kind="ExternalInput")
    dmaskT = nc.dram_tensor("dmaskT", [128, 128], BF16, kind="ExternalInput")
    rotT = nc.dram_tensor("rotT", [128, 128], BF16, kind="ExternalInput")
    yT = nc.dram_tensor("yT", [D, S], F32, kind="ExternalOutput")

    infos = _group_info(p)
    h2 = HD // 2

    with tile.TileContext(nc) as tc, ExitStack() as octx:
        const = octx.enter_context(tc.tile_pool(name="const", bufs=1))
        ident = const.tile([128, 128], BF16)
        make_identity(nc, ident)
        ones_b = const.tile([128, 128], BF16)
        nc.vector.memset(ones_b, 1.0)
        ones_r = const.tile([128, 128], mybir.dt.float32r)
        nc.vector.tensor_copy(out=ones_r, in_=ones_b)
        eps_q_t = const.tile([128, 1], F32)
        nc.vector.memset(eps_q_t, HD * EPS)
        eps_k_t = const.tile([128, 1], F32)
        nc.vector.memset(eps_k_t, EPS)
        zero_t = const.tile([128, 1], F32)
        nc.vector.memset(zero_t, 0.0)
        dmask_sb = const.tile([128, 128], BF16)
        rot_sb = const.tile([128, 128], BF16)
        nc.scalar.dma_start(out=rot_sb, in_=rotT[:, :])

        # weights + trig on the Activation hwdge queue (x / y use SP's)
        wpool = octx.enter_context(tc.tile_pool(name="w", bufs=1))
        wq_sb = wpool.tile([128, DB, HPC * HD], BF16)
        wk_sb = wpool.tile([128, DB, HD], BF16)
        wv_sb = wpool.tile([128, DB, HD], BF16)
        wo_sb = wpool.tile([128, HPC, D], BF16)
        cq_sb = wpool.tile([128, S], BF16)
        sq_sb = wpool.tile([128, S], BF16)
        ck_sb = wpool.tile([128, S], BF16)
        sk_sb = wpool.tile([128, S], BF16)
        nc.scalar.dma_start(
            out=wq_sb, in_=wqT.rearrange("(kb pp) m -> pp kb m", pp=128))
        nc.scalar.dma_start(
            out=wk_sb, in_=wkT.rearrange("(kb pp) m -> pp kb m", pp=128))
        nc.scalar.dma_start(
            out=wv_sb, in_=wvT.rearrange("(kb pp) m -> pp kb m", pp=128))
        nc.scalar.dma_start(out=cq_sb, in_=cos_q[:, :])
        nc.scalar.dma_start(out=sq_sb, in_=sin_q[:, :])
        nc.scalar.dma_start(out=ck_sb, in_=cos_k[:, :])
        nc.scalar.dma_start(out=sk_sb, in_=sin_k[:, :])
        nc.scalar.dma_start(out=dmask_sb, in_=dmaskT[:, :])
        nc.scalar.dma_start(
            out=wo_sb, in_=woT.rearrange("(hb pp) d -> pp hb d", pp=128))

        persist = octx.enter_context(tc.tile_pool(name="persist", bufs=1))
        qTn = persist.tile([128, HPC, S], BF16)   # [hd, h, tok]
        kTn = persist.tile([128, S], BF16)        # [hd, tok]
        v_all = persist.tile([128, SB, HD], BF16)  # [tok(P), kb, hd]

        # ---------------- Phase 1: transposed projections ------------------
        with tc.tile_pool(name="p1x", bufs=2) as p1x, \
             tc.tile_pool(name="p1", bufs=3) as p1, \
             tc.tile_pool(name="p1v", bufs=2) as p1v, \
             tc.tile_pool(name="proj_ps", bufs=3, space="PSUM") as proj_ps, \
             tc.tile_pool(name="zn_ps", bufs=2, space="PSUM") as zn_ps, \
             tc.tile_pool(name="vt_ps", bufs=1, space="PSUM") as vt_ps:

            def proj_chain(w_sb, h, xg):
                ps = proj_ps.tile([128, GS], F32, tag="proj")
                for kb in range(DB):
                    nc.tensor.matmul(
                        ps, lhsT=w_sb[:, kb, h * HD:(h + 1) * HD],
                        rhs=xg[:, kb, :], start=(kb == 0), stop=(kb == DB - 1))
                return ps

            def norm_rope(src_ps, cosv, sinv, dst, sc, bi):
                """RMSNorm + RoPE in [hd, tok] layout; dst bf16 [128, GS].
                Norm factor r = 1/sqrt(sc*Z + bi) broadcast over partitions
                via all-ones matmul (Z = sum_hd q^2).  rotate_half is a PE
                matmul with a signed permutation matrix (DVE cannot read
                partition-shifted operands)."""
                sq = p1.tile([128, GS], BF16, tag="sq")
                nc.scalar.activation(out=sq, in_=src_ps, func=AF.Square,
                                     bias=zero_t)
                zz = zn_ps.tile([128, GS], F32, tag="zz")
                nc.tensor.matmul(zz, lhsT=ones_b, rhs=sq, start=True, stop=True)
                # r = (sc*Z + bi)^(-1/2) = exp(-0.5*ln(sc*Z + bi)) on Act --
                # ln/exp share one table set; DVE reciprocal is ~3.4us/tile.
                lg = p1.tile([128, GS], F32, tag="lg")
                nc.scalar.activation(out=lg, in_=zz, func=AF.Ln,
                                     bias=bi, scale=sc)
                r = p1.tile([128, GS], F32, tag="r")
                nc.scalar.activation(out=r, in_=lg, func=AF.Exp,
                                     bias=zero_t, scale=-0.5)
                qs = p1.tile([128, GS], BF16, tag="qs")
                nc.scalar.copy(out=qs, in_=src_ps)
                rot = zn_ps.tile([128, GS], F32, tag="rot")
                nc.tensor.matmul(rot, lhsT=rot_sb, rhs=qs,
                                 start=True, stop=True)
                t1 = p1.tile([128, GS], BF16, tag="t1")
                nc.vector.tensor_mul(t1, qs, cosv)
                t2 = p1.tile([128, GS], BF16, tag="t2")
                nc.vector.tensor_mul(t2, rot, sinv)
                cb = p1.tile([128, GS], BF16, tag="cb")
                nc.vector.tensor_add(cb, t1, t2)
                nc.vector.tensor_mul(dst, cb, r)

            for g in range(NG):
                ts = slice(g * GS, (g + 1) * GS)
                xg = p1x.tile([128, DB, GS], BF16, tag="xg")
                if g == 0:
                    # split the first load so the first proj chain starts
                    # as soon as the leading half lands
                    hD = D // 2
                    nc.sync.dma_start(
                        out=xg[:, 0:DB // 2, :],
                        in_=xT[0:hD, ts].rearrange(
                            "(kb pp) t -> pp kb t", pp=128))
                    nc.sync.dma_start(
                        out=xg[:, DB // 2:DB, :],
                        in_=xT[hD:D, ts].rearrange(
                            "(kb pp) t -> pp kb t", pp=128))
                else:
                    nc.sync.dma_start(
                        out=xg,
                        in_=xT[:, ts].rearrange("(kb pp) t -> pp kb t", pp=128))

                # pipelined emission: chain h+1 before norm/rope of chain h
                ps_q = [None] * HPC
                ps_q[0] = proj_chain(wq_sb, 0, xg)
                ps_q[1] = proj_chain(wq_sb, 1, xg)
                norm_rope(ps_q[0], cq_sb[:, ts], sq_sb[:, ts],
                          qTn[:, 0, ts], 1.0, eps_q_t)
                ps_q[2] = proj_chain(wq_sb, 2, xg)
                norm_rope(ps_q[1], cq_sb[:, ts], sq_sb[:, ts],
                          qTn[:, 1, ts], 1.0, eps_q_t)
                ps_q[3] = proj_chain(wq_sb, 3, xg)
                norm_rope(ps_q[2], cq_sb[:, ts], sq_sb[:, ts],
                          qTn[:, 2, ts], 1.0, eps_q_t)
                ps_k = proj_chain(wk_sb, 0, xg)
                norm_rope(ps_q[3], cq_sb[:, ts], sq_sb[:, ts],
                          qTn[:, 3, ts], 1.0, eps_q_t)
                ps_v = proj_chain(wv_sb, 0, xg)
                norm_rope(ps_k, ck_sb[:, ts], sk_sb[:, ts],
                          kTn[:, ts], 1.0 / HD, eps_k_t)

                # v: cast to bf16 + PE transpose into [tok, hd]
                vs = p1v.tile([128, GS], BF16, tag="vs")
                nc.scalar.copy(out=vs, in_=ps_v)
                for j in range(4):
                    vt = vt_ps.tile([128, 64], F32, tag="vt")
                    vt_b = vt.bitcast(BF16)
                    nc.tensor.transpose(
                        vt_b, vs[:, j * 128:(j + 1) * 128], ident)
                    nc.vector.tensor_copy(
                        out=v_all[:, g * 4 + j, :], in_=vt_b)

        # ------------- Phase 2: attention + WO, interleaved ----------------
        with tc.tile_pool(name="p2m", bufs=3) as p2m, \
             tc.tile_pool(name="p2a", bufs=2) as p2a, \
             tc.tile_pool(name="p2e", bufs=2) as p2e, \
             tc.tile_pool(name="p2y", bufs=3) as p2y, \
             tc.tile_pool(name="s_ps", bufs=3, space="PSUM") as s_psp, \
             tc.tile_pool(name="z_ps", bufs=1, space="PSUM") as z_psp, \
             tc.tile_pool(name="av_ps", bufs=2, space="PSUM") as av_psp, \
             tc.tile_pool(name="y_ps", bufs=2, space="PSUM") as y_psp:

            def sc_gen(g, h, expT):
                gmax, sfx, diag = infos[g]
                for kb in range(gmax):
                    a = sfx[kb]
                    sp = s_psp.tile([128, GS], F32, tag="s")
                    nc.tensor.matmul(
                        sp[:, 0:GS - a],
                        lhsT=kTn[:, kb * 128:(kb + 1) * 128],
                        rhs=qTn[:, h, g * GS + a:(g + 1) * GS],
                        start=True, stop=not diag[kb])
                    if diag[kb]:
                        nc.tensor.matmul(sp[:, 0:128], lhsT=dmask_sb,
                                         rhs=ident, start=False, stop=True)
                    nc.scalar.activation(out=expT[:, kb, a:GS],
                                         in_=sp[:, 0:GS - a], func=AF.Exp,
                                         bias=zero_t)
                    yield

            def zav_gen(g, h, expT, attnT):
                gmax, sfx, _ = infos[g]
                # sum exp over k-blocks on DVE (fp32, exact); one all-ones
                # matmul then reduces over partitions -> Z broadcast.
                es = p2m.tile([128, GS], mybir.dt.float32r, tag="es")
                if gmax > 1 and sfx[1] == 0:
                    nc.vector.tensor_add(es, expT[:, 0, :], expT[:, 1, :])
                    k0 = 2
                else:
                    nc.vector.tensor_copy(out=es, in_=expT[:, 0, :])
                    k0 = 1
                for kb in range(k0, gmax):
                    a = sfx[kb]
                    nc.vector.tensor_add(es[:, a:GS], es[:, a:GS],
                                         expT[:, kb, a:GS])
                    if kb % 2 == 0:
                        yield
                zp = z_psp.tile([128, GS], F32, tag="z")
                nc.tensor.matmul(zp, lhsT=ones_r, rhs=es,
                                 start=True, stop=True)
                yield
                ap = av_psp.tile([128, GS], F32, tag="av")
                for kb in range(gmax):
                    a = sfx[kb]
                    nc.tensor.matmul(ap[:, a:GS], lhsT=v_all[:, kb, :],
                                     rhs=expT[:, kb, a:GS],
                                     start=(kb == 0), stop=(kb == gmax - 1))
                    yield
                lz = p2m.tile([128, GS], F32, tag="lz")
                nc.scalar.activation(out=lz, in_=zp, func=AF.Ln,
                                     bias=zero_t)
                rz = p2m.tile([128, GS], F32, tag="rz")
                nc.scalar.activation(out=rz, in_=lz, func=AF.Exp,
                                     bias=zero_t, scale=-1.0)
                nc.vector.tensor_mul(attnT[:, h, :], ap, rz)
                yield

            def wo_gen(g, attnT):
                for db in range(DB):
                    yp = y_psp.tile([128, GS], F32, tag="y")
                    for hb in range(HPC):
                        nc.tensor.matmul(
                            yp, lhsT=wo_sb[:, hb, db * 128:(db + 1) * 128],
                            rhs=attnT[:, hb, :],
                            start=(hb == 0), stop=(hb == HPC - 1))
                    ys = p2y.tile([128, GS], F32, tag="ys")
                    if db % 2 == 0:
                        nc.vector.tensor_copy(out=ys, in_=yp)
                    else:
                        nc.scalar.copy(out=ys, in_=yp)
                    nc.sync.dma_start(
                        out=yT[db * 128:(db + 1) * 128, g * GS:(g + 1) * GS],
                        in_=ys)
                    yield

            def pull(gen, n):
                if gen is None:
                    return None
                for _ in range(n):
                    if next(gen, "END") == "END":
                        return None
                return gen

            def drain(gen):
                if gen is not None:
                    for _ in gen:
                        pass

            wo_bg = None
            for g in range(NG):
                attnT = p2a.tile([128, HPC, GS], BF16, tag="attnT")
                zav_bg = None
                for h in range(HPC):
                    expT = p2e.tile([128, SB, GS], BF16, tag="expT")
                    bg = wo_bg if h == 0 else zav_bg
                    for _ in sc_gen(g, h, expT):
                        bg = pull(bg, 1 if h == 0 else 3)
                    if h == 0:
                        wo_bg = bg      # keep leftover WO for the group tail
                    else:
                        drain(bg)
                    zav_bg = zav_gen(g, h, expT, attnT)
                # group tail: drain zav(h3) interleaved with leftover WO
                while zav_bg is not None or wo_bg is not None:
                    zav_bg = pull(zav_bg, 2)
                    wo_bg = pull(wo_bg, 1)
                wo_bg = wo_gen(g, attnT)
            drain(wo_bg)

    if legalize:
        _legalize_waits(nc)
    return nc


def _prep_inputs(x, cos, sin, wq, wk, wv, wo, q_gamma, k_gamma, p):
    """Build the 8 per-core input maps (all host-side prep is free)."""
    bf = ml_dtypes.bfloat16
    cos2 = np.asarray(cos, np.float32).reshape(S, HD)
    sin2 = np.asarray(sin, np.float32).reshape(S, HD)
    qg = np.asarray(q_gamma, np.float32)
    kg = np.asarray(k_gamma, np.float32)
    hh = HD // 2
    qg_rot = np.concatenate([qg[hh:], qg[:hh]])
    kg_rot = np.concatenate([kg[hh:], kg[:hh]])
    cosqT = np.ascontiguousarray((cos2 * qg).T.astype(bf))
    sinqT = np.ascontiguousarray((sin2 * qg_rot).T.astype(bf))
    coskT = np.ascontiguousarray((cos2 * kg).T.astype(bf))
    sinkT = np.ascontiguousarray((sin2 * kg_rot).T.astype(bf))

    ii = np.arange(128)
    dmask = np.where(ii[:, None] <= ii[None, :], 0.0, NEG).astype(np.float32)
    dmaskT = np.ascontiguousarray(dmask.T.astype(bf))

    # rotate_half as a signed permutation: rot = R @ q with
    # R[d, d+64] = -1 (d < 64), R[d, d-64] = +1 (d >= 64); lhsT = R^T.
    h2 = HD // 2
    R = np.zeros((HD, HD), np.float32)
    for dd in range(h2):
        R[dd, dd + h2] = -1.0
        R[dd + h2, dd] = 1.0
    rotT = np.ascontiguousarray(R.T.astype(bf))

    x = np.asarray(x, np.float32)
    wq = np.asarray(wq, np.float32)
    wk = np.asarray(wk, np.float32)
    wv = np.asarray(wv, np.float32)
    wo = np.asarray(wo, np.float32)

    xTb = [np.ascontiguousarray(x[b].T.astype(bf)) for b in range(B)]
    in_maps = []
    for c in range(N_CORES):
        b, gq = divmod(c, N_CORES // B)
        h0 = gq * HPC
        kv = h0 // (NH // KVH)
        in_maps.append({
            "xT": xTb[b],
            "wqT": np.ascontiguousarray(
                wq[h0 * HD:(h0 + HPC) * HD, :].T.astype(bf)),
            "wkT": np.ascontiguousarray(
                wk[kv * HD:(kv + 1) * HD, :].T.astype(bf)),
            "wvT": np.ascontiguousarray(
                wv[kv * HD:(kv + 1) * HD, :].T.astype(bf)),
            "woT": np.ascontiguousarray(
                wo[:, h0 * HD:(h0 + HPC) * HD].T.astype(bf)),
            "cos_q": cosqT, "sin_q": sinqT,
            "cos_k": coskT, "sin_k": sinkT,
            "dmaskT": dmaskT, "rotT": rotT,
        })
    return in_maps


def _gather(results):
    y = np.zeros((B, S, D), dtype=np.float32)
    for c in range(N_CORES):
        b = c // (N_CORES // B)
        y[b] += results[c]["yT"].T
    return y


def kernel(x, cos, sin, wq, wk, wv, wo, q_gamma, k_gamma, signal_token_num):
    p = int(signal_token_num)
    assert p % 128 == 0 and 0 <= p <= S, f"unsupported signal_token_num {p}"

    nc = build_core_kernel(p)
    in_maps = _prep_inputs(x, cos, sin, wq, wk, wv, wo, q_gamma, k_gamma, p)
    res = run_bass_kernel_spmd(nc, in_maps, list(range(N_CORES)))
    return _gather(res.results)


def _install_ntff_hook():
    """The container's antenv lacks axon_hooks; replicate the boot-time NTFF
    profile hook (ctypes into libaxon_pjrt.so) and register the module."""
    import sys
    import types
    import ctypes
    import contextlib

    if "antenv.axon_hooks" in sys.modules:
        return
    so_path = "/opt/axon/libaxon_pjrt.so"
    lib = ctypes.CDLL(so_path)
    if not hasattr(lib, "axon_start_nrt_profile"):
        return
    lib.axon_start_nrt_profile.argtypes = [
        ctypes.POINTER(ctypes.c_int64), ctypes.c_size_t]
    lib.axon_start_nrt_profile.restype = ctypes.c_int64
    lib.axon_stop_nrt_profile.argtypes = [ctypes.c_char_p]
    lib.axon_stop_nrt_profile.restype = ctypes.c_int64

    @contextlib.contextmanager
    def _hook(output_dir, device_ids):
        import jax
        jax.devices()
        if device_ids:
            ids = (ctypes.c_int64 * len(device_ids))(*device_ids)
            rc = lib.axon_start_nrt_profile(ids, len(device_ids))
        else:
            rc = lib.axon_start_nrt_profile(None, 0)
        if rc != 0:
            raise RuntimeError(f"axon_start_nrt_profile rc={rc}")
        try:
            yield
        finally:
            n = lib.axon_stop_nrt_profile(str(output_dir).encode())
            print(f"profile: {n} file(s) written to {output_dir}")

    import antenv
    mod = types.ModuleType("antenv.axon_hooks")
    mod.get_axon_ntff_profile_hook = lambda: _hook
    mod.set_axon_ntff_profile_hook = lambda h: None
    sys.modules["antenv.axon_hooks"] = mod
    antenv.axon_hooks = mod


def profile_once(inputs):
    """Run once with NTFF tracing; return max per-core exec time in ns."""
    import concourse.bass_utils as bu
    bu.upload_artifacts = lambda tmpdir: ""   # no bucket access here
    _install_ntff_hook()
    p = int(inputs["signal_token_num"])
    nc = build_core_kernel(p)
    in_maps = _prep_inputs(
        inputs["x"], inputs["cos"], inputs["sin"], inputs["wq"], inputs["wk"],
        inputs["wv"], inputs["wo"], inputs["q_gamma"], inputs["k_gamma"], p)
    try:
        res = bu.run_bass_kernel_spmd(nc, in_maps, list(range(N_CORES)),
                                      trace=True,
                                      trace_cores=list(range(N_CORES)))
        return res.exec_time_ns
    except Exception as e:
        print(f"profile failed: {type(e).__name__}: {e}")
        return None
